# revision 3
# baseline (speedup 1.0000x reference)
"""Trainium2 Bass kernel for the vq_codebook / ClusteringLayer problem.

Computes, for inputs [N=200000, D=128] and clusters [K=256, D=128]:
    dist2 = ||x||^2 + ||c||^2 - 2 x.c          (GEMM trick)
    q     = 1 / (1 + dist2)                    (ALPHA=1 -> power term is q**1)
    q     = q / sum_k q                        (row normalize)

Sharding: data-parallel over N across 8 NeuronCores; the [K, D] codebook is
replicated. Everything inside one core:

  per 128-row tile:
    PE   : transpose X tile (identity matmul) -> X^T in PSUM
    ACT  : copy X^T PSUM->SBUF
    PE   : MM1  psum  = X^T.T @ (-2 C^T)       (K=128)
           MM2  psum += ones^T @ (1 + ||c||^2) (K=1 rank-1 broadcast)
    DVE  : x_sq = rowsum(X*X)  (tensor_tensor_reduce)
    ACT  : T = psum + x_sq      (activation Identity, per-partition bias)
    DVE  : q = reciprocal_approx_fast(T)
    DVE  : sums = rowsum(q); rsums = 1/sums
    DVE  : out = q * rsums      (tensor_scalar, per-partition scalar)
"""

import sys

if "/opt/trn_rl_repo" not in sys.path:
    sys.path.insert(0, "/opt/trn_rl_repo")

import numpy as np

N_FULL = 200000
D = 128
K = 256
N_CORES = 8
TILE_P = 128
GROUP = 4  # tiles per DMA group
N_PAD = 200704  # = 8 * 25088 = 8 * 196 * 128
ROWS_PER_CORE = N_PAD // N_CORES  # 25088
TILES_PER_CORE = ROWS_PER_CORE // TILE_P  # 196
GROUPS_PER_CORE = TILES_PER_CORE // GROUP  # 49

_PROGRAM = None


def _build_program_v1():
    import concourse.bass as bass
    import concourse.tile as tile
    from concourse import mybir, bacc

    f32 = mybir.dt.float32

    nc = bacc.Bacc("TRN2", target_bir_lowering=False, debug=False,
                   num_devices=N_CORES)

    x_d = nc.dram_tensor("x", [ROWS_PER_CORE, D], f32, kind="ExternalInput").ap()
    ct_d = nc.dram_tensor("ct", [D, K], f32, kind="ExternalInput").ap()
    csq1_d = nc.dram_tensor("csq1", [1, K], f32, kind="ExternalInput").ap()
    ones_d = nc.dram_tensor("ones", [1, TILE_P], f32, kind="ExternalInput").ap()
    ident_d = nc.dram_tensor("ident", [TILE_P, TILE_P], f32,
                             kind="ExternalInput").ap()
    q_d = nc.dram_tensor("q", [ROWS_PER_CORE, K], f32, kind="ExternalOutput").ap()

    with tile.TileContext(nc) as tc:
        with (
            tc.tile_pool(name="consts", bufs=1) as cpool,
            tc.tile_pool(name="xin", bufs=3) as xin_pool,
            tc.tile_pool(name="xt", bufs=3) as xt_pool,
            tc.tile_pool(name="sq", bufs=2) as sq_pool,
            tc.tile_pool(name="tt", bufs=3) as t_pool,
            tc.tile_pool(name="qq", bufs=3) as q_pool,
            tc.tile_pool(name="qn", bufs=3) as qn_pool,
            tc.tile_pool(name="st", bufs=3) as st_pool,
            tc.tile_pool(name="psum_t", bufs=2, space="PSUM") as pst_pool,
            tc.tile_pool(name="psum_q", bufs=4, space="PSUM") as psq_pool,
        ):
            ct_s = cpool.tile([D, K], f32)
            nc.sync.dma_start(ct_s[:], ct_d[:])
            csq1_s = cpool.tile([1, K], f32)
            nc.sync.dma_start(csq1_s[:], csq1_d[:])
            ones_s = cpool.tile([1, TILE_P], f32)
            nc.sync.dma_start(ones_s[:], ones_d[:])
            ident_s = cpool.tile([TILE_P, TILE_P], f32)
            nc.sync.dma_start(ident_s[:], ident_d[:])

            for g in range(GROUPS_PER_CORE):
                r0 = g * GROUP * TILE_P
                xin_g = xin_pool.tile([TILE_P, GROUP, D], f32)
                nc.sync.dma_start(
                    xin_g[:],
                    x_d[r0:r0 + GROUP * TILE_P, :].rearrange(
                        "(t p) d -> p t d", p=TILE_P),
                )
                qn_g = qn_pool.tile([TILE_P, GROUP, K], f32)

                for t in range(GROUP):
                    xin_t = xin_g[:, t, :]

                    xt_ps = pst_pool.tile([TILE_P, TILE_P], f32)
                    nc.tensor.transpose(xt_ps[:], xin_t, ident_s[:])
                    xt_s = xt_pool.tile([TILE_P, TILE_P], f32)
                    nc.scalar.copy(xt_s[:], xt_ps[:])

                    sq_s = sq_pool.tile([TILE_P, D], f32)
                    xsq_col = st_pool.tile([TILE_P, 1], f32, tag="xsq")
                    nc.vector.affine_mul_reduce(
                        out=sq_s[:], accum_out=xsq_col[:],
                        in0=xin_t, in1=xin_t, scale=1.0, bias=0.0,
                    )

                    q_ps = psq_pool.tile([TILE_P, K], f32)
                    nc.tensor.matmul(q_ps[:], xt_s[:], ct_s[:],
                                     start=True, stop=False)
                    nc.tensor.matmul(q_ps[:], ones_s[:], csq1_s[:],
                                     start=False, stop=True)

                    t_s = t_pool.tile([TILE_P, K], f32)
                    nc.scalar.activation(
                        t_s[:], q_ps[:],
                        mybir.ActivationFunctionType.Identity,
                        bias=xsq_col[:], scale=1.0,
                    )

                    q_s = q_pool.tile([TILE_P, K], f32)
                    nc.vector.reciprocal_approx_fast(q_s[:], t_s[:])

                    sums = st_pool.tile([TILE_P, 1], f32, tag="sums")
                    nc.vector.reduce_sum(sums[:], q_s[:],
                                         axis=mybir.AxisListType.X)
                    rsums = st_pool.tile([TILE_P, 1], f32, tag="rsums")
                    nc.vector.reciprocal(rsums[:], sums[:])

                    nc.vector.tensor_scalar_mul(qn_g[:, t, :], q_s[:], rsums[:])

                nc.sync.dma_start(
                    q_d[r0:r0 + GROUP * TILE_P, :].rearrange(
                        "(t p) c -> p t c", p=TILE_P),
                    qn_g[:],
                )

    nc.compile()
    return nc


def _get_program():
    global _PROGRAM
    if _PROGRAM is None:
        _PROGRAM = _build_program_v1()
    return _PROGRAM


def kernel(inputs: np.ndarray, clusters: np.ndarray) -> np.ndarray:
    from concourse import bass_utils

    inputs = np.ascontiguousarray(inputs, dtype=np.float32)
    clusters = np.ascontiguousarray(clusters, dtype=np.float32)

    nc = _get_program()

    x_pad = np.zeros((N_PAD, D), dtype=np.float32)
    x_pad[:N_FULL] = inputs

    ct = np.ascontiguousarray((-2.0 * clusters.T).astype(np.float32))
    csq1 = (1.0 + np.sum(clusters.astype(np.float64) ** 2, axis=1)).astype(
        np.float32)[None, :]
    ones = np.ones((1, TILE_P), dtype=np.float32)
    ident = np.eye(TILE_P, dtype=np.float32)

    in_maps = []
    for c in range(N_CORES):
        shard = x_pad[c * ROWS_PER_CORE:(c + 1) * ROWS_PER_CORE]
        in_maps.append({
            "x": np.ascontiguousarray(shard),
            "ct": ct,
            "csq1": csq1,
            "ones": ones,
            "ident": ident,
        })

    res = bass_utils.run_bass_kernel_spmd(nc, in_maps,
                                          core_ids=list(range(N_CORES)))
    out = np.concatenate([res.results[c]["q"] for c in range(N_CORES)], axis=0)
    return np.ascontiguousarray(out[:N_FULL])


# revision 11
# speedup vs baseline: 2.0812x; 2.0812x over previous
"""Trainium2 Bass kernel for the vq_codebook / ClusteringLayer problem.

Computes, for inputs [N=200000, D=128] and clusters [K=256, D=128]:
    dist2 = ||x||^2 + ||c||^2 - 2 x.c          (GEMM trick)
    q     = 1 / (1 + dist2)                    (ALPHA=1 -> power term is q**1)
    q     = q / sum_k q                        (row normalize)

Sharding: data-parallel over N across 8 NeuronCores; the [K, D] codebook is
replicated. Everything inside one core:

  per 128-row tile:
    PE   : transpose X tile (identity matmul) -> X^T in PSUM
    ACT  : copy X^T PSUM->SBUF
    PE   : MM1  psum  = X^T.T @ (-2 C^T)       (K=128)
           MM2  psum += ones^T @ (1 + ||c||^2) (K=1 rank-1 broadcast)
    DVE  : x_sq = rowsum(X*X)  (tensor_tensor_reduce)
    ACT  : T = psum + x_sq      (activation Identity, per-partition bias)
    DVE  : q = reciprocal_approx_fast(T)
    DVE  : sums = rowsum(q); rsums = 1/sums
    DVE  : out = q * rsums      (tensor_scalar, per-partition scalar)
"""

import sys

if "/opt/trn_rl_repo" not in sys.path:
    sys.path.insert(0, "/opt/trn_rl_repo")

import numpy as np

N_FULL = 200000
D = 128
K = 256
N_CORES = 8
TILE_P = 128
GROUP = 4  # tiles per DMA group
N_PAD = 200704  # = 8 * 25088 = 8 * 196 * 128
ROWS_PER_CORE = N_PAD // N_CORES  # 25088
TILES_PER_CORE = ROWS_PER_CORE // TILE_P  # 196
GROUPS_PER_CORE = TILES_PER_CORE // GROUP  # 49

_PROGRAM = None


def _build_program_v1():
    import concourse.bass as bass
    import concourse.tile as tile
    from concourse import mybir, bacc

    f32 = mybir.dt.float32
    bf16 = mybir.dt.bfloat16

    nc = bacc.Bacc("TRN2", target_bir_lowering=False, debug=False,
                   num_devices=N_CORES)

    x_d = nc.dram_tensor("x", [ROWS_PER_CORE, D], f32, kind="ExternalInput").ap()
    ct_d = nc.dram_tensor("ct", [D, K], bf16, kind="ExternalInput").ap()
    # split-bf16 rank-2 carrier for (1 + ||c||^2): row0=hi, row1=lo
    csq2_d = nc.dram_tensor("csq2", [2, K], bf16, kind="ExternalInput").ap()
    ones2_d = nc.dram_tensor("ones2", [2, TILE_P], bf16,
                             kind="ExternalInput").ap()
    ident_d = nc.dram_tensor("ident", [TILE_P, TILE_P], f32,
                             kind="ExternalInput").ap()
    q_d = nc.dram_tensor("q", [ROWS_PER_CORE, K], f32, kind="ExternalOutput").ap()

    with tile.TileContext(nc) as tc:
        with (
            tc.tile_pool(name="consts", bufs=1) as cpool,
            tc.tile_pool(name="xin", bufs=3) as xin_pool,
            tc.tile_pool(name="xt", bufs=3) as xt_pool,
            tc.tile_pool(name="sq", bufs=2) as sq_pool,
            tc.tile_pool(name="tt", bufs=3) as t_pool,
            tc.tile_pool(name="qq", bufs=GROUP + 2) as q_pool,
            tc.tile_pool(name="qn", bufs=3) as qn_pool,
            tc.tile_pool(name="st", bufs=3) as st_pool,
            tc.tile_pool(name="psum_t", bufs=2, space="PSUM") as pst_pool,
            tc.tile_pool(name="psum_q", bufs=4, space="PSUM") as psq_pool,
        ):
            ct_s = cpool.tile([D, K], bf16)
            nc.sync.dma_start(ct_s[:], ct_d[:])
            csq2_s = cpool.tile([2, K], bf16)
            nc.sync.dma_start(csq2_s[:], csq2_d[:])
            ones2_s = cpool.tile([2, TILE_P], bf16)
            nc.sync.dma_start(ones2_s[:], ones2_d[:])
            ident_s = cpool.tile([TILE_P, TILE_P], f32)
            nc.sync.dma_start(ident_s[:], ident_d[:])

            for g in range(GROUPS_PER_CORE):
                r0 = g * GROUP * TILE_P
                xin_g = xin_pool.tile([TILE_P, GROUP, D], f32)
                nc.sync.dma_start(
                    xin_g[:],
                    x_d[r0:r0 + GROUP * TILE_P, :].rearrange(
                        "(t p) d -> p t d", p=TILE_P),
                )
                qn_g = qn_pool.tile([TILE_P, GROUP, K], f32)
                sums_g = st_pool.tile([TILE_P, GROUP], f32, tag="sumsg")
                rsums_g = st_pool.tile([TILE_P, GROUP], f32, tag="rsumsg")
                q_tiles = []

                for t in range(GROUP):
                    xin_t = xin_g[:, t, :]

                    xt_ps = pst_pool.tile([TILE_P, TILE_P], f32)
                    nc.tensor.transpose(xt_ps[:], xin_t, ident_s[:])
                    xt_s = xt_pool.tile([TILE_P, TILE_P], bf16)
                    nc.scalar.copy(xt_s[:], xt_ps[:])

                    sq_s = sq_pool.tile([TILE_P, D], f32)
                    xsq_col = st_pool.tile([TILE_P, 1], f32, tag="xsq")
                    nc.vector.affine_mul_reduce(
                        out=sq_s[:], accum_out=xsq_col[:],
                        in0=xin_t, in1=xin_t, scale=1.0, bias=0.0,
                    )

                    q_ps = psq_pool.tile([TILE_P, K], f32)
                    nc.tensor.matmul(q_ps[:], xt_s[:], ct_s[:],
                                     start=True, stop=False)
                    nc.tensor.matmul(q_ps[:], ones2_s[:], csq2_s[:],
                                     start=False, stop=True)

                    t_s = t_pool.tile([TILE_P, K], f32)
                    nc.scalar.activation(
                        t_s[:], q_ps[:],
                        mybir.ActivationFunctionType.Identity,
                        bias=xsq_col[:], scale=1.0,
                    )

                    q_s = q_pool.tile([TILE_P, K], f32)
                    nc.vector.reciprocal_approx_fast(q_s[:], t_s[:])
                    q_tiles.append(q_s)

                    nc.vector.reduce_sum(sums_g[:, t:t + 1], q_s[:],
                                         axis=mybir.AxisListType.X)

                nc.vector.reciprocal(rsums_g[:], sums_g[:])

                for t in range(GROUP):
                    nc.vector.tensor_scalar_mul(qn_g[:, t, :], q_tiles[t][:],
                                                rsums_g[:, t:t + 1])

                nc.sync.dma_start(
                    q_d[r0:r0 + GROUP * TILE_P, :].rearrange(
                        "(t p) c -> p t c", p=TILE_P),
                    qn_g[:],
                )

    nc.compile()
    return nc


def _get_program():
    global _PROGRAM
    if _PROGRAM is None:
        _PROGRAM = _build_program_v1()
    return _PROGRAM


def kernel(inputs: np.ndarray, clusters: np.ndarray) -> np.ndarray:
    from concourse import bass_utils

    inputs = np.ascontiguousarray(inputs, dtype=np.float32)
    clusters = np.ascontiguousarray(clusters, dtype=np.float32)

    nc = _get_program()

    x_pad = np.zeros((N_PAD, D), dtype=np.float32)
    x_pad[:N_FULL] = inputs

    import ml_dtypes

    bf16 = ml_dtypes.bfloat16
    ct = np.ascontiguousarray((-2.0 * clusters.T).astype(bf16))
    csq1 = 1.0 + np.sum(clusters.astype(np.float64) ** 2, axis=1)  # [K] f64
    csq_hi = csq1.astype(bf16)
    csq_lo = (csq1 - csq_hi.astype(np.float64)).astype(bf16)
    csq2 = np.ascontiguousarray(np.stack([csq_hi, csq_lo], axis=0))  # [2, K]
    ones2 = np.ones((2, TILE_P), dtype=bf16)
    ident = np.eye(TILE_P, dtype=np.float32)

    in_maps = []
    for c in range(N_CORES):
        shard = x_pad[c * ROWS_PER_CORE:(c + 1) * ROWS_PER_CORE]
        in_maps.append({
            "x": np.ascontiguousarray(shard),
            "ct": ct,
            "csq2": csq2,
            "ones2": ones2,
            "ident": ident,
        })

    res = bass_utils.run_bass_kernel_spmd(nc, in_maps,
                                          core_ids=list(range(N_CORES)))
    out = np.concatenate([res.results[c]["q"] for c in range(N_CORES)], axis=0)
    return np.ascontiguousarray(out[:N_FULL])


# revision 16
# speedup vs baseline: 2.8533x; 1.3710x over previous
"""Trainium2 Bass kernel for the vq_codebook / ClusteringLayer problem.

Computes, for inputs [N=200000, D=128] and clusters [K=256, D=128]:
    dist2 = ||x||^2 + ||c||^2 - 2 x.c          (GEMM trick)
    q     = 1 / (1 + dist2)                    (ALPHA=1 -> power term is q**1)
    q     = q / sum_k q                        (row normalize)

Sharding: data-parallel over N across 8 NeuronCores; the [K, D] codebook is
replicated. Everything inside one core:

  per 128-row tile:
    PE   : transpose X tile (identity matmul) -> X^T in PSUM
    ACT  : copy X^T PSUM->SBUF
    PE   : MM1  psum  = X^T.T @ (-2 C^T)       (K=128)
           MM2  psum += ones^T @ (1 + ||c||^2) (K=1 rank-1 broadcast)
    DVE  : x_sq = rowsum(X*X)  (tensor_tensor_reduce)
    ACT  : T = psum + x_sq      (activation Identity, per-partition bias)
    DVE  : q = reciprocal_approx_fast(T)
    DVE  : sums = rowsum(q); rsums = 1/sums
    DVE  : out = q * rsums      (tensor_scalar, per-partition scalar)
"""

import sys

if "/opt/trn_rl_repo" not in sys.path:
    sys.path.insert(0, "/opt/trn_rl_repo")

import numpy as np

N_FULL = 200000
D = 128
K = 256
N_CORES = 8
TILE_P = 128
GROUP = 4  # tiles per DMA group
N_PAD = 200704  # = 8 * 25088 = 8 * 196 * 128
ROWS_PER_CORE = N_PAD // N_CORES  # 25088
TILES_PER_CORE = ROWS_PER_CORE // TILE_P  # 196
GROUPS_PER_CORE = TILES_PER_CORE // GROUP  # 49

_PROGRAM = None
_FUSED_OP = None


def _register_fused_op():
    """Custom DVE op: out = recip_1nr(in0 + in1 + s0); accum_out = sum(out).

    in0 = PSUM cross term (-2 x.c), in1 = replicated (1 + ||c||^2) row,
    s0 = per-partition ||x||^2, s1/imm2 = minimax seed pair for a
    bitwise-NOT exponent-flip reciprocal seed plus one Newton step
    (~1.7e-3 max rel err over the value range here).
    """
    global _FUSED_OP
    if _FUSED_OP is not None:
        return _FUSED_OP
    import numpy as np
    from operator import add as _add
    from concourse.dve_spec import Spec, Src0, Src1, C0, C1, C2, Zero, AluOp, Bin
    from concourse import dve_ops

    name = "RECIP1NR_BCS_ACC"
    _t = (Src0 + Src1) + C0
    _ny = Bin(AluOp.BITWISE_NOT, _t, _t)
    _z0 = _ny * C1
    _z1 = _z0 * (C2 - _t * _z0)

    def _ref(in0, in1, c0, c1, c2):
        t = (in0.astype(np.float32) + in1 + c0).astype(np.float32)
        ny = (~t.view(np.int32)).view(np.float32)
        z0 = ny * np.float32(c1)
        b = (z0 * (np.float32(c2) - t * z0)).astype(np.float32)
        return b, b.reshape(b.shape[0], -1).sum(axis=-1, keepdims=True)

    op = dve_ops.DveOp(
        name,
        Spec(body=_z1, accum=_add, accum_init=Zero, reference=_ref),
        subdim=False,
        uops_sha={},
    )
    dve_ops.OPS.append(op)
    dve_ops._SUB_OPCODE_FOR_NAME[name] = (
        dve_ops._CUSTOM_DVE_ROW_BASE + len(dve_ops.OPS) - 1)
    dve_ops.CUSTOM_DVE_SPECS[name] = op.spec

    # pin the uops sha (computed locally; equivalent of test_ops_golden)
    from concourse.dve_spec import lower, _has_src1
    from concourse.dve_uop import DveOpSpec

    for ver in ("v3",):
        s = DveOpSpec(name=name, opcode=dve_ops.get_dve_sub_opcode(name),
                      uops=lower(op.spec, ver=ver), rd1_en=_has_src1(op.spec))
        op.uops_sha[ver] = s.sha(ver)
    _FUSED_OP = op
    return op


RECIP_C1 = -0.23549792
RECIP_C2 = 2.0017324


def _build_program_v1():
    import concourse.bass as bass
    import concourse.tile as tile
    from concourse import mybir, bacc

    fused = _register_fused_op()

    f32 = mybir.dt.float32
    bf16 = mybir.dt.bfloat16

    nc = bacc.Bacc("TRN2", target_bir_lowering=False, debug=False,
                   num_devices=N_CORES)

    x_d = nc.dram_tensor("x", [ROWS_PER_CORE, D], f32, kind="ExternalInput").ap()
    ct_d = nc.dram_tensor("ct", [D, K], bf16, kind="ExternalInput").ap()
    # (1 + ||c||^2) replicated across all 128 partitions
    csqr_d = nc.dram_tensor("csqr", [TILE_P, K], f32, kind="ExternalInput").ap()
    ident_d = nc.dram_tensor("ident", [TILE_P, TILE_P], f32,
                             kind="ExternalInput").ap()
    q_d = nc.dram_tensor("q", [ROWS_PER_CORE, K], f32, kind="ExternalOutput").ap()

    with tile.TileContext(nc) as tc:
        with (
            tc.tile_pool(name="consts", bufs=1) as cpool,
            tc.tile_pool(name="xin", bufs=3) as xin_pool,
            tc.tile_pool(name="xt", bufs=3) as xt_pool,
            tc.tile_pool(name="sq", bufs=2) as sq_pool,
            tc.tile_pool(name="tt", bufs=3) as t_pool,
            tc.tile_pool(name="qq", bufs=GROUP + 2) as q_pool,
            tc.tile_pool(name="qn", bufs=3) as qn_pool,
            tc.tile_pool(name="st", bufs=3) as st_pool,
            tc.tile_pool(name="psum_t", bufs=2, space="PSUM") as pst_pool,
            tc.tile_pool(name="psum_q", bufs=4, space="PSUM") as psq_pool,
        ):
            ct_s = cpool.tile([D, K], bf16)
            nc.sync.dma_start(ct_s[:], ct_d[:])
            csqr_s = cpool.tile([TILE_P, K], f32)
            nc.sync.dma_start(csqr_s[:], csqr_d[:])
            ident_s = cpool.tile([TILE_P, TILE_P], f32)
            nc.sync.dma_start(ident_s[:], ident_d[:])

            for g in range(GROUPS_PER_CORE):
                r0 = g * GROUP * TILE_P
                xin_g = xin_pool.tile([TILE_P, GROUP, D], f32)
                nc.sync.dma_start(
                    xin_g[:],
                    x_d[r0:r0 + GROUP * TILE_P, :].rearrange(
                        "(t p) d -> p t d", p=TILE_P),
                )
                qn_g = qn_pool.tile([TILE_P, GROUP, K], f32)
                sums_g = st_pool.tile([TILE_P, GROUP], f32, tag="sumsg")
                rsums_g = st_pool.tile([TILE_P, GROUP], f32, tag="rsumsg")
                q_tiles = []

                for t in range(GROUP):
                    xin_t = xin_g[:, t, :]

                    xt_ps = pst_pool.tile([TILE_P, TILE_P], f32)
                    nc.tensor.transpose(xt_ps[:], xin_t, ident_s[:])
                    xt_s = xt_pool.tile([TILE_P, TILE_P], bf16)
                    nc.scalar.copy(xt_s[:], xt_ps[:])

                    sq_s = sq_pool.tile([TILE_P, D], f32)
                    xsq_col = st_pool.tile([TILE_P, 1], f32, tag="xsq")
                    nc.vector.affine_mul_reduce(
                        out=sq_s[:], accum_out=xsq_col[:],
                        in0=xin_t, in1=xin_t, scale=1.0, bias=0.0,
                    )

                    q_ps = psq_pool.tile([TILE_P, K], f32)
                    nc.tensor.matmul(q_ps[:], xt_s[:], ct_s[:],
                                     start=True, stop=True)

                    # fused: q = recip_1nr(psum + csqr + xsq); sums = sum(q)
                    q_s = q_pool.tile([TILE_P, K], f32)
                    nc.vector._custom_dve(
                        fused, out=q_s[:], in0=q_ps[:], in1=csqr_s[:],
                        s0=xsq_col[:], s1=RECIP_C1, imm2=RECIP_C2,
                        accum_out=sums_g[:, t:t + 1],
                    )
                    q_tiles.append(q_s)

                nc.vector.reciprocal(rsums_g[:], sums_g[:])

                for t in range(GROUP):
                    nc.vector.tensor_scalar_mul(qn_g[:, t, :], q_tiles[t][:],
                                                rsums_g[:, t:t + 1])

                nc.sync.dma_start(
                    q_d[r0:r0 + GROUP * TILE_P, :].rearrange(
                        "(t p) c -> p t c", p=TILE_P),
                    qn_g[:],
                )

    nc.compile()
    return nc


def _get_program():
    global _PROGRAM
    if _PROGRAM is None:
        _PROGRAM = _build_program_v1()
    return _PROGRAM


def kernel(inputs: np.ndarray, clusters: np.ndarray) -> np.ndarray:
    from concourse import bass_utils

    inputs = np.ascontiguousarray(inputs, dtype=np.float32)
    clusters = np.ascontiguousarray(clusters, dtype=np.float32)

    nc = _get_program()

    x_pad = np.zeros((N_PAD, D), dtype=np.float32)
    x_pad[:N_FULL] = inputs

    import ml_dtypes

    bf16 = ml_dtypes.bfloat16
    ct = np.ascontiguousarray((-2.0 * clusters.T).astype(bf16))
    csq1 = (1.0 + np.sum(clusters.astype(np.float64) ** 2, axis=1)).astype(
        np.float32)  # [K]
    csqr = np.ascontiguousarray(np.broadcast_to(csq1[None, :], (TILE_P, K)))
    ident = np.eye(TILE_P, dtype=np.float32)

    in_maps = []
    for c in range(N_CORES):
        shard = x_pad[c * ROWS_PER_CORE:(c + 1) * ROWS_PER_CORE]
        in_maps.append({
            "x": np.ascontiguousarray(shard),
            "ct": ct,
            "csqr": csqr,
            "ident": ident,
        })

    res = bass_utils.run_bass_kernel_spmd(nc, in_maps,
                                          core_ids=list(range(N_CORES)))
    out = np.concatenate([res.results[c]["q"] for c in range(N_CORES)], axis=0)
    return np.ascontiguousarray(out[:N_FULL])
